# revision 1
# baseline (speedup 1.0000x reference)
"""Trainium2 Bass kernel for nn_CrossAttentionForQA (self-contained).

One transformer cross-attention QA layer: QKV proj -> masked MHA -> out proj
-> add&LN -> FFN(gelu) -> add&LN, for B=8, S=1024, E=1024, H=16, F=4096.

Sharding: data-parallel over batch, one batch element per NeuronCore (8 cores,
no collectives). On-device activations live feature-on-partitions (transposed,
[E, S]); inputs are pre-transposed on the host and the output is transposed
back on the host.

Numerics: bf16 GEMM operands with fp32 PSUM accumulation; softmax without
max-subtraction (scores are provably small for this operator); the pairwise
additive mask am[q]&am[k] is folded into the score GEMM as an extra 32-row
contraction band carrying am/32 x am (exact in bf16); the key mask is an exp
bias of -60 per masked key row; softmax denominators come from an extra
all-ones column in the V stationary operand; LayerNorm stats via ones-matmul
on the tensor engine, accumulated on the fly while residual tiles are
produced; LN affine+cast run on the scalar engine in parallel with the
vector-engine normalize passes. y/y2 residual carriers bounce through DRAM
scratch to keep SBUF pool lifetimes strictly LIFO; h1 stays SBUF-resident.
"""

from contextlib import ExitStack

import numpy as np
import ml_dtypes

import concourse.bass as bass
import concourse.tile as tile
from concourse import bacc, mybir
from concourse.bass_utils import run_bass_kernel_spmd

B, S, E, H, F = 8, 1024, 1024, 16, 4096
HD = E // H          # 64
P = 128
ET = E // P          # 8  E-tiles
FT = F // P          # 32 F-tiles
NH = 512             # matmul free-dim chunk (one PSUM bank of fp32)
EPS = 1e-12
QNEG = -60.0         # exp(score + QNEG) ~ 1e-25: negligible vs denom >= 255,
                     # and score+QNEG stays inside the ScalarE exp LUT range

bf = mybir.dt.bfloat16
f32 = mybir.dt.float32
AF = mybir.ActivationFunctionType
OP = mybir.AluOpType
bf16np = ml_dtypes.bfloat16

_CACHE: dict = {}


def _build(nc: bass.Bass):
    # ---------------- DRAM parameters (per core) ----------------
    xT_d = nc.declare_dram_parameter("xT", [E, S], f32, False)       # x transposed
    xTb_d = nc.declare_dram_parameter("xTb", [E, S], bf, False)      # x^T in bf16
    w1_d = nc.declare_dram_parameter("w1", [E, 3 * E], bf, False)    # q-part /8
    wo_d = nc.declare_dram_parameter("wo", [E, E], bf, False)
    win_d = nc.declare_dram_parameter("win", [E, F], bf, False)
    wout_d = nc.declare_dram_parameter("wout", [F, E], bf, False)
    amb_d = nc.declare_dram_parameter("amb", [S], bf, False)         # attn mask 0/1
    amc_d = nc.declare_dram_parameter("amc", [S], bf, False)         # am / 32
    bvb_d = nc.declare_dram_parameter("bvb", [P, E], f32, False)     # v-bias bcast
    ppq_d = nc.declare_dram_parameter("ppq", [P, ET], f32, False)    # b1 q-part /8
    ppk_d = nc.declare_dram_parameter("ppk", [P, ET], f32, False)    # b1 k-part
    ppo_d = nc.declare_dram_parameter("ppo", [P, ET], f32, False)    # out_proj_b
    ppi_d = nc.declare_dram_parameter("ppi", [P, FT], f32, False)    # b_in
    ppu_d = nc.declare_dram_parameter("ppu", [P, ET], f32, False)    # b_out
    ppw_d = nc.declare_dram_parameter("ppw", [P, ET], f32, False)    # ln_w
    ppb_d = nc.declare_dram_parameter("ppb", [P, ET], f32, False)    # ln_b
    ppm_d = nc.declare_dram_parameter("ppm", [P, ET], f32, False)    # key-mask bias
    out_d = nc.declare_dram_parameter("outT", [E, S], f32, True)

    # DRAM scratch for the first residual carrier (y2 stays SBUF-resident)
    yf_d = nc.dram_tensor("yf_s", [E, S], f32)

    def r3(d):  # [E,S] dram -> [P, ET, S] tiled view
        return d.rearrange("(t p) s -> p t s", p=P)

    # small DRAM scratch rows used to broadcast a [1, S] vector across
    # partitions (DMA out, then DMA back with a partition-broadcast view;
    # SBUF APs cannot partition-broadcast but DRAM APs can)
    bscr = [nc.dram_tensor(f"bscr{i}", [S], f32) for i in range(4)]
    _bn = [0]

    def bcast(src_row, dst_ap, rows):
        scr = bscr[_bn[0] % len(bscr)]
        _bn[0] += 1
        nc.sync.dma_start(scr[None, :], src_row)
        nc.sync.dma_start(dst_ap, scr[None, :].broadcast_to([rows, S]))

    with tile.TileContext(nc) as tc:
        with ExitStack() as root:
            const = root.enter_context(tc.tile_pool(name="const", bufs=1))
            mmp = root.enter_context(tc.tile_pool(name="mmp", bufs=2, space="PSUM"))
            ctxp = root.enter_context(tc.tile_pool(name="ctxp", bufs=2, space="PSUM"))

            # ------------- constants -------------
            ppq = const.tile([P, ET], f32, tag="ppq")
            ppk = const.tile([P, ET], f32, tag="ppk")
            ppo = const.tile([P, ET], f32, tag="ppo")
            ppi = const.tile([P, FT], f32, tag="ppi")
            ppu = const.tile([P, ET], f32, tag="ppu")
            ppw = const.tile([P, ET], f32, tag="ppw")
            ppb = const.tile([P, ET], f32, tag="ppb")
            ppm = const.tile([P, ET], f32, tag="ppm")
            bvbs = const.tile([P, E], f32, tag="bvbs")
            onesml = const.tile([P, 2], bf, tag="ones")  # col0: 1/1024
            epst = const.tile([1, 1], f32, tag="eps")
            for tt, dd in ((ppq, ppq_d), (ppk, ppk_d), (ppo, ppo_d), (ppi, ppi_d),
                           (ppu, ppu_d), (ppw, ppw_d), (ppb, ppb_d), (ppm, ppm_d),
                           (bvbs, bvb_d)):
                nc.sync.dma_start(tt[:], dd[:])
            nc.vector.memset(onesml[:, 0:1], 1.0 / 1024.0)
            nc.vector.memset(onesml[:, 1:2], 1.0)
            nc.vector.memset(epst[:], float(EPS))

            def stats_mm(yb, idx, mups, eyps):
                """Accumulate mu/E[y^2] for one [P, S] bf16 tile of y.
                Squares yb in place after the mu pass consumed it."""
                for half in range(2):
                    nc.tensor.matmul(
                        mups[:, half * NH:(half + 1) * NH],
                        lhsT=onesml[:, 0:1],
                        rhs=yb[:, half * NH:(half + 1) * NH],
                        start=(idx == 0), stop=(idx == ET - 1),
                    )
                nc.scalar.activation(yb[:], yb[:], AF.Square)
                for half in range(2):
                    nc.tensor.matmul(
                        eyps[:, half * NH:(half + 1) * NH],
                        lhsT=onesml[:, 0:1],
                        rhs=yb[:, half * NH:(half + 1) * NH],
                        start=(idx == 0), stop=(idx == ET - 1),
                    )

            with tc.tile_pool(name="pctx", bufs=1) as pctx, \
                 tc.tile_pool(name="pout", bufs=2) as pout:
                ctxT = pctx.tile([P, ET, S], bf, tag="ctxT")
                with tc.tile_pool(name="pqkv", bufs=1) as pqkv:
                    qhat = pqkv.tile([P, H, S], bf, tag="qhat")
                    khat = pqkv.tile([P, H, S], bf, tag="khat")
                    vhat = pqkv.tile([P, ET, H, HD + 1], bf, tag="vhat")

                    # ---- phase 1: QKV projections ----
                    with tc.tile_pool(name="pw1", bufs=1) as pw1:
                        xbf = pw1.tile([P, ET, S], bf, tag="xbf")
                        w1s = pw1.tile([P, ET, 3 * E], bf, tag="w1s")
                        with tc.high_priority():
                            for kt in range(ET):
                                nc.sync.dma_start(
                                    xbf[:, kt, :], r3(xTb_d)[:, kt, :]
                                )
                                nc.sync.dma_start(
                                    w1s[:, kt, :],
                                    w1_d.rearrange("(t p) f -> p t f", p=P)[:, kt, :],
                                )

                        # q^T, k^T: [feat_tile, sq] = W.T @ x
                        for tf in range(2 * ET):
                            isq = tf < ET
                            t = tf % ET
                            foff = t * P if isq else E + t * P
                            ps = mmp.tile([P, S], f32, tag="mm")
                            for half in range(2):
                                for kt in range(ET):
                                    nc.tensor.matmul(
                                        ps[:, half * NH:(half + 1) * NH],
                                        lhsT=w1s[:, kt, foff:foff + P],
                                        rhs=xbf[:, kt, half * NH:(half + 1) * NH],
                                        start=(kt == 0),
                                        stop=(kt == ET - 1),
                                    )
                            dst = qhat if isq else khat
                            pp = ppq if isq else ppk
                            nc.vector.tensor_scalar_add(
                                dst[0:HD, 2 * t, :], ps[0:HD, :], pp[0:HD, t:t + 1]
                            )
                            nc.vector.tensor_scalar_add(
                                dst[HD:P, 2 * t + 1, :], ps[HD:P, :], pp[HD:P, t:t + 1]
                            )

                        # mask bands / zero padding (needed from attention on;
                        # emitted here so their DMAs don't compete with the
                        # startup weight loads). Head parity layout per
                        # [128, S] block (all partition bases 32-aligned):
                        # the pairwise mask am[q]&am[k] enters the score
                        # contraction via a 32-row band am/32 (qhat) x am
                        # (khat): 32*(am/32)*am = am*am, exact in bf16.
                        #   even head: data 0:64, band 64:96, zeros 96:128
                        #   odd head:  zeros 0:32, band 32:64, data 64:128
                        for t, band in ((qhat, amc_d), (khat, amb_d)):
                            ev = t.rearrange("p (hp two) s -> p hp two s", two=2)
                            nc.vector.memset(ev[96:P, :, 0, :], 0.0)
                            nc.vector.memset(ev[0:32, :, 1, :], 0.0)
                            nc.sync.dma_start(
                                ev[64:96, :, 0, :],
                                band[None, None, :].broadcast_to([32, H // 2, S]),
                            )
                            nc.sync.dma_start(
                                ev[32:64, :, 1, :],
                                band[None, None, :].broadcast_to([32, H // 2, S]),
                            )
                        nc.vector.memset(vhat[:, :, :, HD:HD + 1], 1.0)

                        # v natural: [sq_tile, feat] = x @ Wv
                        for st in range(ET):
                            ps = mmp.tile([P, E], f32, tag="mm")
                            for half in range(2):
                                for kt in range(ET):
                                    nc.tensor.matmul(
                                        ps[:, half * NH:(half + 1) * NH],
                                        lhsT=xbf[:, kt, st * P:(st + 1) * P],
                                        rhs=w1s[:, kt,
                                                2 * E + half * NH:
                                                2 * E + (half + 1) * NH],
                                        start=(kt == 0),
                                        stop=(kt == ET - 1),
                                    )
                            nc.vector.tensor_tensor(
                                vhat[:, st, :, 0:HD],
                                ps.rearrange("p (h d) -> p h d", d=HD),
                                bvbs.rearrange("p (h d) -> p h d", d=HD),
                                OP.add,
                            )

                    # ---- phase 2: attention ----
                    # odd head first within each pair so the final normalize
                    # tail (which gates out-proj) is an even head with no
                    # extra ctxT DMA hop
                    head_order = []
                    for hp in range(H // 2):
                        head_order += [2 * hp + 1, 2 * hp]
                    with tc.tile_pool(name="patt", bufs=2) as attw:
                        for h in head_order:
                            cx = ctxp.tile([P, S], f32, tag="ctx")
                            for skt in range(ET):
                                sc = mmp.tile([P, S], f32, tag="mm")
                                for half in range(2):
                                    nc.tensor.matmul(
                                        sc[:, half * NH:(half + 1) * NH],
                                        lhsT=khat[:, h, skt * P:(skt + 1) * P],
                                        rhs=qhat[:, h, half * NH:(half + 1) * NH],
                                        start=True,
                                        stop=True,
                                    )
                                pb = attw.tile([P, S], bf, tag="probs", bufs=3)
                                nc.scalar.activation(
                                    pb[:], sc[:], AF.Exp, bias=ppm[:, skt:skt + 1]
                                )
                                for half in range(2):
                                    nc.tensor.matmul(
                                        cx[0:HD + 1, half * NH:(half + 1) * NH],
                                        lhsT=vhat[:, skt, h, :],
                                        rhs=pb[:, half * NH:(half + 1) * NH],
                                        start=(skt == 0),
                                        stop=(skt == ET - 1),
                                    )
                            # rows 0:64 = ctx_u, row 64 = softmax denominator
                            rc = attw.tile([P, S], f32, tag="rc")
                            nc.vector.reciprocal(rc[HD:HD + 1, :], cx[HD:HD + 1, :])
                            rb = attw.tile([P, S], f32, tag="rb")
                            bcast(rc[HD:HD + 1, :], rb[0:HD, :], HD)
                            if h % 2 == 0:
                                nc.vector.tensor_tensor(
                                    ctxT[0:HD, h // 2, :], cx[0:HD, :], rb[0:HD, :],
                                    OP.mult,
                                )
                            else:
                                tmp = attw.tile([HD, S], bf, tag="octx")
                                nc.vector.tensor_tensor(
                                    tmp[:], cx[0:HD, :], rb[0:HD, :], OP.mult
                                )
                                nc.sync.dma_start(ctxT[HD:P, h // 2, :], tmp[:])

                # ---- phase 3: out proj (-> y to DRAM, stats on the fly) ----
                mups = ctxp.tile([1, S], f32, tag="ctx")
                eyps = ctxp.tile([1, S], f32, tag="ctx")
                for ft in range(ET):
                    wt = pout.tile([P, ET, P], bf, tag="wo", bufs=2)
                    nc.sync.dma_start(
                        wt[:],
                        wo_d.rearrange("(t p) f -> p t f", p=P)[
                            :, :, ft * P:(ft + 1) * P
                        ],
                    )
                    ps = mmp.tile([P, S], f32, tag="mm")
                    for half in range(2):
                        for kt in range(ET):
                            nc.tensor.matmul(
                                ps[:, half * NH:(half + 1) * NH],
                                lhsT=wt[:, kt, :],
                                rhs=ctxT[:, kt, half * NH:(half + 1) * NH],
                                start=(kt == 0),
                                stop=(kt == ET - 1),
                            )
                    tv = pout.tile([P, S], f32, tag="tv")
                    nc.scalar.activation(
                        tv[:], ps[:], AF.Identity, bias=ppo[:, ft:ft + 1]
                    )
                    xt = pout.tile([P, S], f32, tag="xt")
                    nc.sync.dma_start(xt[:], r3(xT_d)[:, ft, :])
                    yt = pout.tile([P, S], f32, tag="yt")
                    nc.vector.tensor_tensor(yt[:], tv[:], xt[:], OP.add)
                    nc.sync.dma_start(r3(yf_d)[:, ft, :], yt[:])
                    yb = pout.tile([P, S], bf, tag="yb", bufs=2)
                    nc.vector.tensor_copy(out=yb[:], in_=yt[:])
                    stats_mm(yb, ft, mups, eyps)

            # ---- LN1 -> h1 (SBUF); FFN; GEMM2 stats; LN2 -> out ----
            py2 = root.enter_context(tc.tile_pool(name="py2", bufs=1))
            y2f = py2.tile([P, ET, S], f32, tag="y2f")
            with tc.tile_pool(name="pg", bufs=1) as pg:
                gT = pg.tile([P, FT, S], bf, tag="gT")
                with tc.tile_pool(name="ph1f", bufs=1) as ph1f:
                    h1f = ph1f.tile([P, ET, S], f32, tag="h1f")
                    with tc.tile_pool(name="ph1b", bufs=1) as ph1b:
                        h1bf = ph1b.tile([P, ET, S], bf, tag="h1bf")

                        _ln_normalize(nc, tc, const, mups, eyps, yf_d,
                                      None, h1f, h1bf, bcast, epst, ppw, ppb, r3)

                        # FFN GEMM1 + gelu
                        for ftile in range(FT):
                            wt = ph1b.tile([P, ET, P], bf, tag="win", bufs=3)
                            nc.sync.dma_start(
                                wt[:],
                                win_d.rearrange("(t p) f -> p t f", p=P)[
                                    :, :, ftile * P:(ftile + 1) * P
                                ],
                            )
                            ps = mmp.tile([P, S], f32, tag="mm")
                            for half in range(2):
                                for kt in range(ET):
                                    nc.tensor.matmul(
                                        ps[:, half * NH:(half + 1) * NH],
                                        lhsT=wt[:, kt, :],
                                        rhs=h1bf[:, kt, half * NH:(half + 1) * NH],
                                        start=(kt == 0),
                                        stop=(kt == ET - 1),
                                    )
                            nc.scalar.activation(
                                gT[:, ftile, :], ps[:], AF.Gelu,
                                bias=ppi[:, ftile:ftile + 1],
                            )

                    # FFN GEMM2 (-> y2 SBUF, stats on the fly)
                    mups2 = ctxp.tile([1, S], f32, tag="ctx")
                    eyps2 = ctxp.tile([1, S], f32, tag="ctx")
                    with tc.tile_pool(name="pg2", bufs=2) as pg2:
                        for et in range(ET):
                            wt2 = pg2.tile([P, FT, P], bf, tag="wout", bufs=2)
                            nc.sync.dma_start(
                                wt2[:],
                                wout_d.rearrange("(t p) f -> p t f", p=P)[
                                    :, :, et * P:(et + 1) * P
                                ],
                            )
                            ps = mmp.tile([P, S], f32, tag="mm")
                            for half in range(2):
                                for kt in range(FT):
                                    nc.tensor.matmul(
                                        ps[:, half * NH:(half + 1) * NH],
                                        lhsT=wt2[:, kt, :],
                                        rhs=gT[:, kt, half * NH:(half + 1) * NH],
                                        start=(kt == 0),
                                        stop=(kt == FT - 1),
                                    )
                            tv = pg2.tile([P, S], f32, tag="tv")
                            nc.scalar.activation(
                                tv[:], ps[:], AF.Identity, bias=ppu[:, et:et + 1]
                            )
                            nc.vector.tensor_tensor(
                                y2f[:, et, :], tv[:], h1f[:, et, :], OP.add
                            )
                            yb = pg2.tile([P, S], bf, tag="yb", bufs=2)
                            nc.vector.tensor_copy(out=yb[:], in_=y2f[:, et, :])
                            stats_mm(yb, et, mups2, eyps2)

            _ln_normalize(nc, tc, const, mups2, eyps2, y2f, out_d, None, None,
                          bcast, epst, ppw, ppb, r3, src_sb=True)

    return nc


def _ln_normalize(nc, tc, const, mups, eyps, src_d, dst_d, hf, hbf, bcast,
                  epst, ppw, ppb, r3, src_sb=False):
    """Finish LN given accumulated stats psums: compute mu/rstd, broadcast,
    stream src tiles back and write the normalized result.

    DVE does (y - mu_b) * r_b; ACT applies the per-feature affine (and the
    bf16 cast) in parallel. Output goes to dst_d (DRAM fp32) or to hf/hbf
    SBUF tiles.
    """
    mu = const.tile([1, S], f32, tag="mu")
    rr = const.tile([1, S], f32, tag="rr")
    nc.vector.tensor_copy(out=mu[:], in_=mups[:])
    nc.vector.tensor_tensor(rr[:], mu[:], mu[:], OP.mult)
    nc.vector.tensor_tensor(rr[:], eyps[:], rr[:], OP.subtract)
    nc.scalar.activation(rr[:], rr[:], AF.Sqrt, bias=epst[:])
    nc.vector.reciprocal(rr[:], rr[:])
    with tc.tile_pool(name="pln", bufs=2) as pln:
        mub = pln.tile([P, S], f32, tag="mub", bufs=1)
        rb2 = pln.tile([P, S], f32, tag="rb2", bufs=1)
        bcast(mu[:], mub[:], P)
        bcast(rr[:], rb2[:], P)
        for t in range(ET):
            if src_sb:
                yt = src_d[:, t, :]
            else:
                yt = pln.tile([P, S], f32, tag="ys", bufs=3)
                nc.sync.dma_start(yt[:], r3(src_d)[:, t, :])
            tv = pln.tile([P, S], f32, tag="lt")
            nc.vector.tensor_tensor(tv[:], yt[:], mub[:], OP.subtract)
            nc.vector.tensor_tensor(tv[:], tv[:], rb2[:], OP.mult)
            if hf is not None:
                nc.scalar.activation(
                    hf[:, t, :], tv[:], AF.Identity,
                    bias=ppb[:, t:t + 1], scale=ppw[:, t:t + 1],
                )
                nc.scalar.activation(hbf[:, t, :], hf[:, t, :], AF.Identity)
            else:
                ov = pln.tile([P, S], f32, tag="ov")
                nc.scalar.activation(
                    ov[:], tv[:], AF.Identity,
                    bias=ppb[:, t:t + 1], scale=ppw[:, t:t + 1],
                )
                nc.sync.dma_start(r3(dst_d)[:, t, :], ov[:])


def get_nc():
    if "nc" not in _CACHE:
        # Bacc (not plain Bass): its compile() pass splits semaphore waits to
        # the TRN2 limit of one wait per instruction (generate_event_semaphores)
        nc = bacc.Bacc("TRN2")
        _build(nc)
        nc.finalize()
        _CACHE["nc"] = nc
    return _CACHE["nc"]


def _strided_pp(v: np.ndarray) -> np.ndarray:
    """[n*128] feature vector -> [128, n] per-partition layout (col t = tile t)."""
    return np.ascontiguousarray(v.reshape(-1, P).T.astype(np.float32))


def make_in_maps(inputs: dict) -> list[dict]:
    x = np.asarray(inputs["final_hidden_state"], np.float32)
    am_i = np.asarray(inputs["attention_mask"]) != 0
    tt = np.asarray(inputs["token_type_ids"])

    w1 = np.array(np.asarray(inputs["in_proj_w"], np.float32))
    b1 = np.array(np.asarray(inputs["in_proj_b"], np.float32))
    w1[:, 0:E] /= 8.0
    b1q = b1[0:E] / 8.0

    shared = {
        "w1": w1.astype(bf16np),
        "wo": np.asarray(inputs["out_proj_w"], np.float32).astype(bf16np),
        "win": np.asarray(inputs["w_in"], np.float32).astype(bf16np),
        "wout": np.asarray(inputs["w_out"], np.float32).astype(bf16np),
        "ppq": _strided_pp(b1q),
        "ppk": _strided_pp(b1[E:2 * E]),
        "ppo": _strided_pp(np.asarray(inputs["out_proj_b"], np.float32)),
        "ppi": _strided_pp(np.asarray(inputs["b_in"], np.float32)),
        "ppu": _strided_pp(np.asarray(inputs["b_out"], np.float32)),
        "ppw": _strided_pp(np.asarray(inputs["ln_w"], np.float32)),
        "ppb": _strided_pp(np.asarray(inputs["ln_b"], np.float32)),
        "bvb": np.ascontiguousarray(
            np.broadcast_to(b1[2 * E:3 * E][None, :], (P, E)).astype(np.float32)
        ),
    }
    qm = (tt == 1) | (~am_i)
    qm[:, 0] = True
    maps = []
    for b in range(B):
        m = dict(shared)
        xT = np.ascontiguousarray(x[b].T)
        m["xT"] = xT
        m["xTb"] = xT.astype(bf16np)
        m["amb"] = am_i[b].astype(bf16np)
        m["amc"] = (am_i[b].astype(np.float32) / 32.0).astype(bf16np)
        m["ppm"] = _strided_pp(np.where(qm[b], np.float32(QNEG), np.float32(0.0)))
        maps.append(m)
    return maps


def run(inputs: dict, trace: bool = False):
    nc = get_nc()
    res = run_bass_kernel_spmd(nc, make_in_maps(inputs), list(range(B)), trace=trace)
    out = np.stack([np.asarray(r["outT"], np.float32).T for r in res.results])
    return out, res


def kernel(**inputs) -> np.ndarray:
    out, _ = run(inputs)
    return out



# revision 3
# speedup vs baseline: 3.9904x; 3.9904x over previous
"""Trainium2 Bass kernel for nn_CrossAttentionForQA (self-contained).

One transformer cross-attention QA layer: QKV proj -> masked MHA -> out proj
-> add&LN -> FFN(gelu) -> add&LN, for B=8, S=1024, E=1024, H=16, F=4096.

Sharding: data-parallel over batch, one batch element per NeuronCore (8 cores).
Weights are NOT duplicated over the host link: each core uploads a 1/8 row
shard of each weight matrix (3 MB/core instead of 24 MB/core) and the full
matrices are reassembled on-device with NeuronLink AllGathers into Shared
DRAM. Activations live feature-on-partitions (transposed, [E, S]); x is
pre-transposed on the host (bf16), and the output is returned transposed
fp16 and undone on the host. All small per-feature constants travel in one
packed [128, 88] fp32 parameter.

Numerics: bf16 GEMM operands with fp32 PSUM accumulation; softmax without
max-subtraction (scores are provably small for this operator); the pairwise
additive mask am[q]&am[k] is folded into the score GEMM as an extra 32-row
contraction band carrying am/32 x am (exact in bf16); the key mask is an exp
bias of -60 per masked key row; softmax denominators come from an extra
all-ones column in the V stationary operand; LayerNorm stats via ones-matmul
on the tensor engine, accumulated on the fly while residual tiles are
produced; LN affine+cast run on the scalar engine in parallel with the
vector-engine normalize passes. y/y2 residual carriers bounce through DRAM
scratch to keep SBUF pool lifetimes strictly LIFO; h1 stays SBUF-resident.
"""

from contextlib import ExitStack

import numpy as np
import ml_dtypes

import concourse.bass as bass
import concourse.tile as tile
from concourse import bacc, mybir
from concourse.bass_utils import run_bass_kernel_spmd

B, S, E, H, F = 8, 1024, 1024, 16, 4096
HD = E // H          # 64
P = 128
ET = E // P          # 8  E-tiles
FT = F // P          # 32 F-tiles
NH = 512             # matmul free-dim chunk (one PSUM bank of fp32)
EPS = 1e-12
QNEG = -60.0         # exp(score + QNEG) ~ 1e-25: negligible vs denom >= 255,
                     # and score+QNEG stays inside the ScalarE exp LUT range

# column bases inside the packed [P, 88] constant parameter
Q0, K0, O0, I0, U0, W0, B0, M0 = 0, 8, 16, 24, 56, 64, 72, 80
PPC = 88

bf = mybir.dt.bfloat16
f16 = mybir.dt.float16
f32 = mybir.dt.float32
AF = mybir.ActivationFunctionType
OP = mybir.AluOpType
bf16np = ml_dtypes.bfloat16

_CACHE: dict = {}


def _build(nc: bass.Bass):
    # ---------------- DRAM parameters (per core) ----------------
    xTb_d = nc.declare_dram_parameter("xTb", [E, S], bf, False)      # x^T in bf16
    w1s_d = nc.declare_dram_parameter("w1s", [P, 3 * E], bf, False)  # row shards
    wos_d = nc.declare_dram_parameter("wos", [P, E], bf, False)
    wins_d = nc.declare_dram_parameter("wins", [P, F], bf, False)
    wouts_d = nc.declare_dram_parameter("wouts", [F // B, E], bf, False)
    amb_d = nc.declare_dram_parameter("amb", [S], bf, False)         # attn mask 0/1
    amc_d = nc.declare_dram_parameter("amc", [S], bf, False)         # am / 32
    pps_d = nc.declare_dram_parameter("pps", [P, PPC], f32, False)   # packed biases
    bv_d = nc.declare_dram_parameter("bv", [E], f32, False)          # v bias
    out_d = nc.declare_dram_parameter("outT", [E, S], f16, True)

    # full weights, reassembled on-device from the per-core shards
    w1g = nc.dram_tensor("w1g", [E, 3 * E], bf, addr_space="Shared")
    wog = nc.dram_tensor("wog", [E, E], bf, addr_space="Shared")
    wing = nc.dram_tensor("wing", [E, F], bf, addr_space="Shared")
    woutg = nc.dram_tensor("woutg", [F, E], bf, addr_space="Shared")
    # collectives can't read I/O tensors directly -> Local DRAM bounce
    ag = [
        (w1s_d, nc.dram_tensor("w1_bnc", [P, 3 * E], bf), w1g),
        (wos_d, nc.dram_tensor("wo_bnc", [P, E], bf), wog),
        (wins_d, nc.dram_tensor("win_bnc", [P, F], bf), wing),
        (wouts_d, nc.dram_tensor("wout_bnc", [F // B, E], bf), woutg),
    ]

    # DRAM scratch for the first residual carrier (y2 stays SBUF-resident)
    yf_d = nc.dram_tensor("yf_s", [E, S], f32)

    def r3(d):  # [E,S] dram -> [P, ET, S] tiled view
        return d.rearrange("(t p) s -> p t s", p=P)

    # small DRAM scratch rows used to broadcast a [1, S] vector across
    # partitions (DMA out, then DMA back with a partition-broadcast view;
    # SBUF APs cannot partition-broadcast but DRAM APs can)
    bscr = [nc.dram_tensor(f"bscr{i}", [S], f32) for i in range(4)]
    _bn = [0]

    def bcast(src_row, dst_ap, rows):
        scr = bscr[_bn[0] % len(bscr)]
        _bn[0] += 1
        nc.sync.dma_start(scr[None, :], src_row)
        nc.sync.dma_start(dst_ap, scr[None, :].broadcast_to([rows, S]))

    with tile.TileContext(nc) as tc:
        # reassemble full weights first (gpsimd queue; compute DMAs overlap)
        for src, bnc, full in ag:
            nc.gpsimd.dma_start(out=bnc[:, :], in_=src[:, :])
            nc.gpsimd.collective_compute(
                "AllGather",
                mybir.AluOpType.bypass,
                replica_groups=[list(range(B))],
                ins=[bnc.ap().opt()],
                outs=[full.ap().opt()],
            )

        with ExitStack() as root:
            const = root.enter_context(tc.tile_pool(name="const", bufs=1))
            mmp = root.enter_context(tc.tile_pool(name="mmp", bufs=2, space="PSUM"))
            ctxp = root.enter_context(tc.tile_pool(name="ctxp", bufs=2, space="PSUM"))

            # ------------- constants -------------
            pps = const.tile([P, PPC], f32, tag="pps")
            bvbs = const.tile([P, E], f32, tag="bvbs")
            onesml = const.tile([P, 2], bf, tag="ones")  # col0: 1/1024
            epst = const.tile([1, 1], f32, tag="eps")
            nc.sync.dma_start(pps[:], pps_d[:])
            nc.sync.dma_start(bvbs[:], bv_d[None, :].broadcast_to([P, E]))
            nc.vector.memset(onesml[:, 0:1], 1.0 / 1024.0)
            nc.vector.memset(onesml[:, 1:2], 1.0)
            nc.vector.memset(epst[:], float(EPS))

            def stats_mm(yb, idx, mups, eyps):
                """Accumulate mu/E[y^2] for one [P, S] bf16 tile of y.
                Squares yb in place after the mu pass consumed it."""
                for half in range(2):
                    nc.tensor.matmul(
                        mups[:, half * NH:(half + 1) * NH],
                        lhsT=onesml[:, 0:1],
                        rhs=yb[:, half * NH:(half + 1) * NH],
                        start=(idx == 0), stop=(idx == ET - 1),
                    )
                nc.scalar.activation(yb[:], yb[:], AF.Square)
                for half in range(2):
                    nc.tensor.matmul(
                        eyps[:, half * NH:(half + 1) * NH],
                        lhsT=onesml[:, 0:1],
                        rhs=yb[:, half * NH:(half + 1) * NH],
                        start=(idx == 0), stop=(idx == ET - 1),
                    )

            with tc.tile_pool(name="pctx", bufs=1) as pctx, \
                 tc.tile_pool(name="pout", bufs=2) as pout:
                ctxT = pctx.tile([P, ET, S], bf, tag="ctxT")
                with tc.tile_pool(name="pqkv", bufs=1) as pqkv:
                    qhat = pqkv.tile([P, H, S], bf, tag="qhat")
                    khat = pqkv.tile([P, H, S], bf, tag="khat")
                    vhat = pqkv.tile([P, ET, H, HD + 1], bf, tag="vhat")

                    # ---- phase 1: QKV projections ----
                    with tc.tile_pool(name="pw1", bufs=1) as pw1:
                        xbf = pw1.tile([P, ET, S], bf, tag="xbf")
                        w1s = pw1.tile([P, ET, 3 * E], bf, tag="w1s")
                        with tc.high_priority():
                            for kt in range(ET):
                                nc.sync.dma_start(
                                    xbf[:, kt, :], r3(xTb_d)[:, kt, :]
                                )
                                nc.sync.dma_start(
                                    w1s[:, kt, :],
                                    w1g.rearrange("(t p) f -> p t f", p=P)[:, kt, :],
                                )

                        # q^T, k^T: [feat_tile, sq] = W.T @ x
                        for tf in range(2 * ET):
                            isq = tf < ET
                            t = tf % ET
                            foff = t * P if isq else E + t * P
                            ps = mmp.tile([P, S], f32, tag="mm")
                            for half in range(2):
                                for kt in range(ET):
                                    nc.tensor.matmul(
                                        ps[:, half * NH:(half + 1) * NH],
                                        lhsT=w1s[:, kt, foff:foff + P],
                                        rhs=xbf[:, kt, half * NH:(half + 1) * NH],
                                        start=(kt == 0),
                                        stop=(kt == ET - 1),
                                    )
                            dst = qhat if isq else khat
                            base = Q0 if isq else K0
                            nc.vector.tensor_scalar_add(
                                dst[0:HD, 2 * t, :], ps[0:HD, :],
                                pps[0:HD, base + t:base + t + 1]
                            )
                            nc.vector.tensor_scalar_add(
                                dst[HD:P, 2 * t + 1, :], ps[HD:P, :],
                                pps[HD:P, base + t:base + t + 1]
                            )

                        # mask bands / zero padding (needed from attention on;
                        # emitted here so their DMAs don't compete with the
                        # startup weight loads). Head parity layout per
                        # [128, S] block (all partition bases 32-aligned):
                        # the pairwise mask am[q]&am[k] enters the score
                        # contraction via a 32-row band am/32 (qhat) x am
                        # (khat): 32*(am/32)*am = am*am, exact in bf16.
                        #   even head: data 0:64, band 64:96, zeros 96:128
                        #   odd head:  zeros 0:32, band 32:64, data 64:128
                        for t, band in ((qhat, amc_d), (khat, amb_d)):
                            ev = t.rearrange("p (hp two) s -> p hp two s", two=2)
                            nc.vector.memset(ev[96:P, :, 0, :], 0.0)
                            nc.vector.memset(ev[0:32, :, 1, :], 0.0)
                            nc.sync.dma_start(
                                ev[64:96, :, 0, :],
                                band[None, None, :].broadcast_to([32, H // 2, S]),
                            )
                            nc.sync.dma_start(
                                ev[32:64, :, 1, :],
                                band[None, None, :].broadcast_to([32, H // 2, S]),
                            )
                        nc.vector.memset(vhat[:, :, :, HD:HD + 1], 1.0)

                        # v natural: [sq_tile, feat] = x @ Wv
                        for st in range(ET):
                            ps = mmp.tile([P, E], f32, tag="mm")
                            for half in range(2):
                                for kt in range(ET):
                                    nc.tensor.matmul(
                                        ps[:, half * NH:(half + 1) * NH],
                                        lhsT=xbf[:, kt, st * P:(st + 1) * P],
                                        rhs=w1s[:, kt,
                                                2 * E + half * NH:
                                                2 * E + (half + 1) * NH],
                                        start=(kt == 0),
                                        stop=(kt == ET - 1),
                                    )
                            nc.vector.tensor_tensor(
                                vhat[:, st, :, 0:HD],
                                ps.rearrange("p (h d) -> p h d", d=HD),
                                bvbs.rearrange("p (h d) -> p h d", d=HD),
                                OP.add,
                            )

                    # ---- phase 2: attention ----
                    # odd head first within each pair so the final normalize
                    # tail (which gates out-proj) is an even head with no
                    # extra ctxT DMA hop
                    head_order = []
                    for hp in range(H // 2):
                        head_order += [2 * hp + 1, 2 * hp]
                    with tc.tile_pool(name="patt", bufs=2) as attw:
                        for h in head_order:
                            cx = ctxp.tile([P, S], f32, tag="ctx")
                            for skt in range(ET):
                                sc = mmp.tile([P, S], f32, tag="mm")
                                for half in range(2):
                                    nc.tensor.matmul(
                                        sc[:, half * NH:(half + 1) * NH],
                                        lhsT=khat[:, h, skt * P:(skt + 1) * P],
                                        rhs=qhat[:, h, half * NH:(half + 1) * NH],
                                        start=True,
                                        stop=True,
                                    )
                                pb = attw.tile([P, S], bf, tag="probs", bufs=3)
                                nc.scalar.activation(
                                    pb[:], sc[:], AF.Exp,
                                    bias=pps[:, M0 + skt:M0 + skt + 1]
                                )
                                for half in range(2):
                                    nc.tensor.matmul(
                                        cx[0:HD + 1, half * NH:(half + 1) * NH],
                                        lhsT=vhat[:, skt, h, :],
                                        rhs=pb[:, half * NH:(half + 1) * NH],
                                        start=(skt == 0),
                                        stop=(skt == ET - 1),
                                    )
                            # rows 0:64 = ctx_u, row 64 = softmax denominator
                            rc = attw.tile([P, S], f32, tag="rc")
                            nc.vector.reciprocal(rc[HD:HD + 1, :], cx[HD:HD + 1, :])
                            rb = attw.tile([P, S], f32, tag="rb")
                            bcast(rc[HD:HD + 1, :], rb[0:HD, :], HD)
                            if h % 2 == 0:
                                nc.vector.tensor_tensor(
                                    ctxT[0:HD, h // 2, :], cx[0:HD, :], rb[0:HD, :],
                                    OP.mult,
                                )
                            else:
                                tmp = attw.tile([HD, S], bf, tag="octx")
                                nc.vector.tensor_tensor(
                                    tmp[:], cx[0:HD, :], rb[0:HD, :], OP.mult
                                )
                                nc.sync.dma_start(ctxT[HD:P, h // 2, :], tmp[:])

                # ---- phase 3: out proj (-> y to DRAM, stats on the fly) ----
                mups = ctxp.tile([1, S], f32, tag="ctx")
                eyps = ctxp.tile([1, S], f32, tag="ctx")
                for ft in range(ET):
                    wt = pout.tile([P, ET, P], bf, tag="wo", bufs=2)
                    nc.sync.dma_start(
                        wt[:],
                        wog.rearrange("(t p) f -> p t f", p=P)[
                            :, :, ft * P:(ft + 1) * P
                        ],
                    )
                    ps = mmp.tile([P, S], f32, tag="mm")
                    for half in range(2):
                        for kt in range(ET):
                            nc.tensor.matmul(
                                ps[:, half * NH:(half + 1) * NH],
                                lhsT=wt[:, kt, :],
                                rhs=ctxT[:, kt, half * NH:(half + 1) * NH],
                                start=(kt == 0),
                                stop=(kt == ET - 1),
                            )
                    tv = pout.tile([P, S], f32, tag="tv")
                    nc.scalar.activation(
                        tv[:], ps[:], AF.Identity, bias=pps[:, O0 + ft:O0 + ft + 1]
                    )
                    xt = pout.tile([P, S], bf, tag="xt")
                    nc.sync.dma_start(xt[:], r3(xTb_d)[:, ft, :])
                    yt = pout.tile([P, S], f32, tag="yt")
                    nc.vector.tensor_tensor(yt[:], tv[:], xt[:], OP.add)
                    nc.sync.dma_start(r3(yf_d)[:, ft, :], yt[:])
                    yb = pout.tile([P, S], bf, tag="yb", bufs=2)
                    nc.vector.tensor_copy(out=yb[:], in_=yt[:])
                    stats_mm(yb, ft, mups, eyps)

            # ---- LN1 -> h1 (SBUF); FFN; GEMM2 stats; LN2 -> out ----
            py2 = root.enter_context(tc.tile_pool(name="py2", bufs=1))
            y2f = py2.tile([P, ET, S], f32, tag="y2f")
            with tc.tile_pool(name="pg", bufs=1) as pg:
                gT = pg.tile([P, FT, S], bf, tag="gT")
                with tc.tile_pool(name="ph1f", bufs=1) as ph1f:
                    h1f = ph1f.tile([P, ET, S], f32, tag="h1f")
                    with tc.tile_pool(name="ph1b", bufs=1) as ph1b:
                        h1bf = ph1b.tile([P, ET, S], bf, tag="h1bf")

                        _ln_normalize(nc, tc, const, mups, eyps, yf_d,
                                      None, h1f, h1bf, bcast, epst, pps, r3)

                        # FFN GEMM1 + gelu
                        for ftile in range(FT):
                            wt = ph1b.tile([P, ET, P], bf, tag="win", bufs=3)
                            nc.sync.dma_start(
                                wt[:],
                                wing.rearrange("(t p) f -> p t f", p=P)[
                                    :, :, ftile * P:(ftile + 1) * P
                                ],
                            )
                            ps = mmp.tile([P, S], f32, tag="mm")
                            for half in range(2):
                                for kt in range(ET):
                                    nc.tensor.matmul(
                                        ps[:, half * NH:(half + 1) * NH],
                                        lhsT=wt[:, kt, :],
                                        rhs=h1bf[:, kt, half * NH:(half + 1) * NH],
                                        start=(kt == 0),
                                        stop=(kt == ET - 1),
                                    )
                            nc.scalar.activation(
                                gT[:, ftile, :], ps[:], AF.Gelu,
                                bias=pps[:, I0 + ftile:I0 + ftile + 1],
                            )

                    # FFN GEMM2 (-> y2 SBUF, stats on the fly)
                    mups2 = ctxp.tile([1, S], f32, tag="ctx")
                    eyps2 = ctxp.tile([1, S], f32, tag="ctx")
                    with tc.tile_pool(name="pg2", bufs=2) as pg2:
                        for et in range(ET):
                            wt2 = pg2.tile([P, FT, P], bf, tag="wout", bufs=2)
                            nc.sync.dma_start(
                                wt2[:],
                                woutg.rearrange("(t p) f -> p t f", p=P)[
                                    :, :, et * P:(et + 1) * P
                                ],
                            )
                            ps = mmp.tile([P, S], f32, tag="mm")
                            for half in range(2):
                                for kt in range(FT):
                                    nc.tensor.matmul(
                                        ps[:, half * NH:(half + 1) * NH],
                                        lhsT=wt2[:, kt, :],
                                        rhs=gT[:, kt, half * NH:(half + 1) * NH],
                                        start=(kt == 0),
                                        stop=(kt == FT - 1),
                                    )
                            tv = pg2.tile([P, S], f32, tag="tv")
                            nc.scalar.activation(
                                tv[:], ps[:], AF.Identity,
                                bias=pps[:, U0 + et:U0 + et + 1]
                            )
                            nc.vector.tensor_tensor(
                                y2f[:, et, :], tv[:], h1f[:, et, :], OP.add
                            )
                            yb = pg2.tile([P, S], bf, tag="yb", bufs=2)
                            nc.vector.tensor_copy(out=yb[:], in_=y2f[:, et, :])
                            stats_mm(yb, et, mups2, eyps2)

            _ln_normalize(nc, tc, const, mups2, eyps2, y2f, out_d, None, None,
                          bcast, epst, pps, r3, src_sb=True)

    return nc


def _ln_normalize(nc, tc, const, mups, eyps, src_d, dst_d, hf, hbf, bcast,
                  epst, pps, r3, src_sb=False):
    """Finish LN given accumulated stats psums: compute mu/rstd, broadcast,
    stream src tiles back and write the normalized result.

    DVE does (y - mu_b) * r_b; ACT applies the per-feature affine (and the
    dtype cast) in parallel. Output goes to dst_d (DRAM fp16) or to hf/hbf
    SBUF tiles.
    """
    mu = const.tile([1, S], f32, tag="mu")
    rr = const.tile([1, S], f32, tag="rr")
    nc.vector.tensor_copy(out=mu[:], in_=mups[:])
    nc.vector.tensor_tensor(rr[:], mu[:], mu[:], OP.mult)
    nc.vector.tensor_tensor(rr[:], eyps[:], rr[:], OP.subtract)
    nc.scalar.activation(rr[:], rr[:], AF.Sqrt, bias=epst[:])
    nc.vector.reciprocal(rr[:], rr[:])
    with tc.tile_pool(name="pln", bufs=2) as pln:
        mub = pln.tile([P, S], f32, tag="mub", bufs=1)
        rb2 = pln.tile([P, S], f32, tag="rb2", bufs=1)
        bcast(mu[:], mub[:], P)
        bcast(rr[:], rb2[:], P)
        for t in range(ET):
            if src_sb:
                yt = src_d[:, t, :]
            else:
                yt = pln.tile([P, S], f32, tag="ys", bufs=3)
                nc.sync.dma_start(yt[:], r3(src_d)[:, t, :])
            tv = pln.tile([P, S], f32, tag="lt")
            nc.vector.tensor_tensor(tv[:], yt[:], mub[:], OP.subtract)
            nc.vector.tensor_tensor(tv[:], tv[:], rb2[:], OP.mult)
            if hf is not None:
                nc.scalar.activation(
                    hf[:, t, :], tv[:], AF.Identity,
                    bias=pps[:, B0 + t:B0 + t + 1], scale=pps[:, W0 + t:W0 + t + 1],
                )
                nc.scalar.activation(hbf[:, t, :], hf[:, t, :], AF.Identity)
            else:
                ov = pln.tile([P, S], f16, tag="ov")
                nc.scalar.activation(
                    ov[:], tv[:], AF.Identity,
                    bias=pps[:, B0 + t:B0 + t + 1], scale=pps[:, W0 + t:W0 + t + 1],
                )
                nc.sync.dma_start(r3(dst_d)[:, t, :], ov[:])


def get_nc():
    if "nc" not in _CACHE:
        # Bacc (not plain Bass): its compile() pass splits semaphore waits to
        # the TRN2 limit of one wait per instruction (generate_event_semaphores)
        nc = bacc.Bacc("TRN2", num_devices=B)
        _build(nc)
        nc.finalize()
        _CACHE["nc"] = nc
    return _CACHE["nc"]


def _strided_pp(v: np.ndarray) -> np.ndarray:
    """[n*128] feature vector -> [128, n] per-partition layout (col t = tile t)."""
    return np.ascontiguousarray(v.reshape(-1, P).T.astype(np.float32))


def make_in_maps(inputs: dict) -> list[dict]:
    x = np.asarray(inputs["final_hidden_state"], np.float32)
    am_i = np.asarray(inputs["attention_mask"]) != 0
    tt = np.asarray(inputs["token_type_ids"])

    w1 = np.array(np.asarray(inputs["in_proj_w"], np.float32))
    b1 = np.asarray(inputs["in_proj_b"], np.float32)
    w1[:, 0:E] /= 8.0
    b1q = b1[0:E] / 8.0

    w1b = w1.astype(bf16np)
    wob = np.asarray(inputs["out_proj_w"], np.float32).astype(bf16np)
    winb = np.asarray(inputs["w_in"], np.float32).astype(bf16np)
    woutb = np.asarray(inputs["w_out"], np.float32).astype(bf16np)

    # packed [P, 80] shared constant block (ppm appended per core -> [P, 88])
    pps_shared = np.concatenate([
        _strided_pp(b1q),                                        # Q0
        _strided_pp(b1[E:2 * E]),                                # K0
        _strided_pp(np.asarray(inputs["out_proj_b"], np.float32)),   # O0
        _strided_pp(np.asarray(inputs["b_in"], np.float32)),     # I0
        _strided_pp(np.asarray(inputs["b_out"], np.float32)),    # U0
        _strided_pp(np.asarray(inputs["ln_w"], np.float32)),     # W0
        _strided_pp(np.asarray(inputs["ln_b"], np.float32)),     # B0
    ], axis=1)
    bv = np.ascontiguousarray(b1[2 * E:3 * E])

    qm = (tt == 1) | (~am_i)
    qm[:, 0] = True
    maps = []
    FS = F // B
    for b in range(B):
        ppm = _strided_pp(np.where(qm[b], np.float32(QNEG), np.float32(0.0)))
        maps.append({
            "xTb": x[b].T.astype(bf16np),
            "w1s": np.ascontiguousarray(w1b[b * P:(b + 1) * P]),
            "wos": np.ascontiguousarray(wob[b * P:(b + 1) * P]),
            "wins": np.ascontiguousarray(winb[b * P:(b + 1) * P]),
            "wouts": np.ascontiguousarray(woutb[b * FS:(b + 1) * FS]),
            "amb": am_i[b].astype(bf16np),
            "amc": (am_i[b].astype(np.float32) / 32.0).astype(bf16np),
            "pps": np.ascontiguousarray(
                np.concatenate([pps_shared, ppm], axis=1)
            ),
            "bv": bv,
        })
    return maps


def run(inputs: dict, trace: bool = False):
    nc = get_nc()
    res = run_bass_kernel_spmd(nc, make_in_maps(inputs), list(range(B)), trace=trace)
    out = np.stack(
        [np.asarray(r["outT"]).astype(np.float32).T for r in res.results]
    )
    return out, res


def kernel(**inputs) -> np.ndarray:
    out, _ = run(inputs)
    return out
